# revision 12
# baseline (speedup 1.0000x reference)
"""Block-sparse linear kernel for Trainium2 — 32x32 PE-tiled sparse matmul.

out = x @ W.T + bias, W 4096x4096 with 8192 active 32x32 blocks (50% dense).

Per core (tokens sharded 8 ways -> 1024 tokens/core, all blocks):
  - TensorE in 32x32 tiling mode: 16 independent sub-arrays (r, c).
  - Block (mb, kb): col strip c = mb%4, row strip r = kb%4, job J = mb//4.
  - Per block: 1 LDWEIGHTS + 2 MMs (token chunks nn0/nn1 of 512); the
    second MM reuses loaded weights via InstMatmult.ldweights=False.
  - PSUM bank map: nn0 chains -> pb[r], nn1 chains -> pb[4+r]; the 4 col
    strips share banks at disjoint partition ranges (start=True clear is
    range-local on TRN2, verified).
  - Per job J (4 m-blocks, one per strip): DVE merges r-banks, r0/r1
    banks first so their tiles can start job J+1 early; bias folded via
    scalar_tensor_tensor; -> SBUF -> DMA out.
  - Weights streamed through a 2-chunk SBUF ring in schedule order; x
    (8MB fp16) resident.
"""

import numpy as np

import concourse.bacc as bacc
import concourse.mybir as mybir
from concourse.bass_utils import run_bass_kernel_spmd

F16 = mybir.dt.float16
F32 = mybir.dt.float32

TOKENS = 8192
IN = 4096
OUT = 4096
BS = 32
NCORES = 8
TPC = TOKENS // NCORES   # 1024
NCH = 512
NJ = 32                  # jobs (128 m-blocks / 4 strips)
WCHUNK = 512             # W ring chunk, q-items per strip
WRING = 2 * WCHUNK
XQ = 4                   # x DMA quarters

_CACHE: dict = {}


def _schedule(block_rows, block_cols):
    chains = [[[[] for _ in range(4)] for _ in range(4)] for _ in range(NJ)]
    for b in range(len(block_rows)):
        mb = int(block_rows[b])
        kb = int(block_cols[b])
        chains[mb // 4][mb % 4][kb % 4].append((kb // 4, b))
    for J in range(NJ):
        for c in range(4):
            for r in range(4):
                chains[J][c][r].sort()
    return chains


def _emission(chains):
    """Paced emission order. r0/r1 chains finish ~68%, r3 ~85%, r2 last;
    r2 items start late so job J+1's r2 MMs clear the evac latency.

    emitted[i]: dict(J, r, c, F, b, q, start, stop, wait_a, wait_2, wait_3,
                     last_a, last_3, last, pw_incs)
    """
    F_R = {0: 1.5, 1: 1.5, 2: 1.0, 3: 1.2}
    emitted = []
    qctr = [0, 0, 0, 0]
    for J in range(NJ):
        tilework = {}
        for c in range(4):
            for r in range(4):
                items = list(chains[J][c][r]) or [(0, -1)]
                tilework[(r, c)] = items
        ptr = {k: 0 for k in tilework}
        total = sum(len(v) for v in tilework.values())
        done = 0
        jitems = []
        first = {0: True, 1: True, 2: True, 3: True}
        last_idx = {0: None, 1: None, 2: None, 3: None}
        while done < total:
            best = None
            bestv = None
            for r in range(4):
                for c in range(4):
                    items = tilework[(r, c)]
                    p = ptr[(r, c)]
                    if p >= len(items):
                        continue
                    if r == 2 and done < 0.24 * total:
                        continue
                    v = (p + 1) / (len(items) * F_R[r])
                    if bestv is None or v < bestv:
                        bestv = v
                        best = (r, c)
            if best is None:  # only r2 left but gated -> release gate
                for r in (2,):
                    for c in range(4):
                        if ptr[(r, c)] < len(tilework[(r, c)]):
                            best = (r, c)
                            break
                    if best:
                        break
            r, c = best
            p = ptr[(r, c)]
            F, b = tilework[(r, c)][p]
            ptr[(r, c)] = p + 1
            done += 1
            q = qctr[r]
            qctr[r] = q + 1
            e = dict(J=J, r=r, c=c, F=F, b=b, q=q,
                     start=(p == 0), stop=(p == len(tilework[(r, c)]) - 1),
                     wait_a=(first[0] and J > 0 and r < 2),
                     wait_2=(first[2] and J > 0 and r == 2),
                     wait_3=(first[3] and J > 0 and r == 3),
                     last_a=False, last_3=False, last=False, pw_incs=())
            if r < 2:
                first[0] = first[1] = False
            else:
                first[r] = False
            jitems.append(e)
        # move the job's final r2 item to the very end so the job-completion
        # flags (last / last_3 / last_a) land on three distinct items.
        k2 = max(i for i, e in enumerate(jitems) if e["r"] == 2)
        jitems.append(jitems.pop(k2))
        for i, e in enumerate(jitems):
            if e["r"] < 2:
                last_idx[0] = i
            else:
                last_idx[e["r"]] = i
        jitems[last_idx[0]]["last_a"] = True
        jitems[last_idx[3]]["last_3"] = True
        jitems[-1]["last"] = True
        emitted.extend(jitems)

    nq = list(qctr)
    nchunks = (max(nq) + WCHUNK - 1) // WCHUNK
    prog = [0, 0, 0, 0]
    cdone = set()
    chunk_done_job = {}
    for idx, e in enumerate(emitted):
        prog[e["r"]] = e["q"] + 1
        for k in range(nchunks):
            if k in cdone:
                continue
            if all(prog[r] >= min((k + 1) * WCHUNK, nq[r]) for r in range(4)):
                cdone.add(k)
                e["pw_incs"] = e["pw_incs"] + (k,)
                chunk_done_job[k] = e["J"]
    return emitted, nq, nchunks, chunk_done_job


def _build(emitted, nchunks, chunk_done_job):
    nc = bacc.Bacc("TRN2", target_bir_lowering=False, debug=False)
    wcols = nchunks * WCHUNK * 32

    xt = nc.dram_tensor("xt", [128, 32 * TPC], F16, kind="ExternalInput")
    wimg = nc.dram_tensor("wimg", [128, wcols], F16, kind="ExternalInput")
    bias_img = nc.dram_tensor("bias_img", [128, NJ], F32, kind="ExternalInput")
    outT = nc.dram_tensor("outT", [OUT, TPC], F32, kind="ExternalOutput")

    from contextlib import ExitStack

    with ExitStack() as st:
        sem = {n: st.enter_context(nc.semaphore(n))
               for n in ("xsem", "xsem2", "wsem", "bsem", "pwsem", "pja", "pj3", "pjb",
                         "aca", "evsa", "evsb", "evsc", "evout", "odsem",
                         "gsem")}
        xsem, xsem2, wsem, bsem, pwsem = (sem["xsem"], sem["xsem2"],
                                          sem["bsem"], sem["bsem"],
                                          sem["pwsem"])
        xsem, xsem2, wsem, bsem = (sem["xsem"], sem["xsem2"], sem["wsem"],
                                   sem["bsem"])
        pja, pj3, pjb, aca = sem["pja"], sem["pj3"], sem["pjb"], sem["aca"]
        evsa, evsb, evsc = sem["evsa"], sem["evsb"], sem["evsc"]
        evout, odsem, gsem = sem["evout"], sem["odsem"], sem["gsem"]
        xb = st.enter_context(nc.sbuf_tensor("xb", [128, 32 * TPC], F16))
        wring = st.enter_context(nc.sbuf_tensor("wring", [128, WRING * 32], F16))
        warm = st.enter_context(nc.sbuf_tensor("warm", [128, 128], F16))
        biast = st.enter_context(nc.sbuf_tensor("biast", [128, NJ], F32))
        tmps = [st.enter_context(nc.sbuf_tensor(f"tmp{i}", [128, NCH], F32))
                for i in range(4)]
        scp = [st.enter_context(nc.sbuf_tensor(f"scp{i}", [128, NCH], F32))
               for i in range(4)]   # ACT copies of pb1, pb5, pb3, pb7
        obufs = [st.enter_context(nc.sbuf_tensor(f"ob{i}", [128, TPC], F32))
                 for i in range(2)]
        pb = [st.enter_context(nc.psum_tensor(f"pb{i}", [128, NCH], F32))
              for i in range(8)]

        # sync-ring W chunk placement: chunk k (k>=2) waits pwsem >= k-1,
        # which fires at job chunk_done_job[k-2]; place it after that job's
        # out-DMA.  chunk 1 goes up front (no wait).
        wplace = {}
        for k in range(2, nchunks):
            wplace.setdefault(chunk_done_job[k - 2], []).append(k)

        with nc.Block() as block:

            @block.gpsimd
            def _(gpsimd):
                gpsimd.memset(warm.ap(), 0.0).then_inc(gsem, 1)

            @block.scalar
            def _(scalar):
                ccols = WCHUNK * 32
                qs = (32 * TPC) // XQ
                for k in range(XQ):
                    scalar.dma_start(xb.ap()[:, k * qs:(k + 1) * qs],
                                     xt.ap()[:, k * qs:(k + 1) * qs]).then_inc(
                        xsem, 16)
                if nchunks > 1:
                    scalar.dma_start(
                        wring.ap()[:, ccols:2 * ccols],
                        wimg.ap()[:, ccols:2 * ccols]).then_inc(wsem, 16)
                for J in range(NJ):
                    for k in wplace.get(J, ()):
                        scalar.wait_ge(pwsem, k - 1)
                        reg = (k % 2) * ccols
                        scalar.dma_start(
                            wring.ap()[:, reg:reg + ccols],
                            wimg.ap()[:, k * ccols:(k + 1) * ccols]).then_inc(
                            wsem, 16)
                    scalar.wait_ge(pja, J + 1)
                    scalar.copy(scp[0].ap(), pb[1].ap())
                    scalar.copy(scp[1].ap(), pb[5].ap()).then_inc(aca, 1)
                    scalar.wait_ge(pj3, J + 1)
                    scalar.copy(scp[2].ap(), pb[3].ap())
                    scalar.copy(scp[3].ap(), pb[7].ap()).then_inc(evsc, 1)

            @block.sync
            def _(sync):
                ccols = WCHUNK * 32
                sync.dma_start(wring.ap()[:, 0:ccols],
                               wimg.ap()[:, 0:ccols]).then_inc(wsem, 16)
                sync.dma_start(biast.ap(), bias_img.ap()).then_inc(bsem, 16)
                for J in range(NJ):
                    sync.wait_ge(evout, J + 1)
                    sync.dma_start(outT.ap()[128 * J:128 * J + 128, :],
                                   obufs[J % 2].ap()).then_inc(odsem, 16)
                sync.wait_ge(odsem, 16 * NJ)

            @block.tensor
            def _(tensor):
                tensor.wait_ge(gsem, 1)
                for i in range(160):
                    tensor.matmul(pb[i % 8].ap()[:, 0:64], warm.ap(),
                                  warm.ap()[:, 0:64], start=True, stop=True,
                                  skip_group_check=True)
                xwait = 0
                wwait = 0
                pending = []
                for e in emitted:
                    J, r, c, F, q = e["J"], e["r"], e["c"], e["F"], e["q"]
                    need_xq = F // (32 // XQ) + 1
                    if need_xq > xwait:
                        xwait = need_xq
                        tensor.wait_ge(xsem, 16 * xwait)
                    need_wc = q // WCHUNK + 1
                    if need_wc > wwait:
                        wwait = need_wc
                        tensor.wait_ge(wsem, 16 * wwait)
                    if e["wait_a"]:
                        tensor.wait_ge(evsa, J)
                    if e["wait_2"]:
                        tensor.wait_ge(evsb, J)
                    if e["wait_3"]:
                        tensor.wait_ge(evsc, J)
                    roff = (q % WRING) * 32
                    w = wring.ap()[32 * r:32 * r + 32, roff:roff + 32]
                    x0 = xb.ap()[32 * r:32 * r + 32, F * TPC:F * TPC + NCH]
                    x1 = xb.ap()[32 * r:32 * r + 32,
                                 F * TPC + NCH:F * TPC + 2 * NCH]
                    p0 = pb[r].ap()[32 * c:32 * c + 32, :]
                    p1 = pb[4 + r].ap()[32 * c:32 * c + 32, :]
                    m0 = tensor.matmul(p0, w, x0, start=e["start"],
                                       stop=e["stop"],
                                       tile_position=(32 * r, 32 * c),
                                       skip_group_check=True)
                    m1 = tensor.matmul(p1, w, x1, start=e["start"],
                                       stop=e["stop"],
                                       tile_position=(32 * r, 32 * c),
                                       skip_group_check=True)
                    m1.ins.ldweights = False
                    # slot assignment: each MM carries at most one sem update;
                    # spill extra updates to later items (strictly-later inc
                    # is always safe for >= waits).
                    slots = []
                    if not e["pw_incs"]:
                        slots.append(m0)
                    else:
                        m0.then_inc(pwsem, len(e["pw_incs"]))
                    slots.append(m1)
                    for flag, sm in (("last", pjb), ("last_3", pj3),
                                     ("last_a", pja)):
                        if e[flag]:
                            pending.append(sm)
                    while pending and slots:
                        slots.pop(-1).then_inc(pending.pop(0), 1)

            @block.vector
            def _(vector):
                vector.wait_ge(bsem, 16)
                for J in range(NJ):
                    if J >= 2:
                        vector.wait_ge(odsem, 16 * (J - 1))
                    ob = obufs[J % 2]
                    bj = biast.ap()[:, J:J + 1]
                    vector.wait_ge(aca, J + 1)
                    vector.tensor_add(tmps[0].ap(), scp[0].ap(), pb[0].ap())
                    vector.tensor_add(tmps[1].ap(), scp[1].ap(),
                                      pb[4].ap()).then_inc(evsa, 1)
                    vector.wait_ge(evsc, J + 1)
                    vector.wait_ge(pjb, J + 1)
                    vector.tensor_add(tmps[2].ap(), scp[2].ap(), pb[2].ap())
                    vector.tensor_add(tmps[3].ap(), scp[3].ap(),
                                      pb[6].ap()).then_inc(evsb, 1)
                    vector.scalar_tensor_tensor(
                        ob.ap()[:, 0:NCH], tmps[0].ap(), bj, tmps[2].ap(),
                        op0=mybir.AluOpType.add, op1=mybir.AluOpType.add)
                    vector.scalar_tensor_tensor(
                        ob.ap()[:, NCH:2 * NCH], tmps[1].ap(), bj,
                        tmps[3].ap(), op0=mybir.AluOpType.add,
                        op1=mybir.AluOpType.add).then_inc(evout, 1)

    nc.compile()
    return nc


def _pack(x, weight_data, bias, emitted, nq, nchunks):
    wd = np.asarray(weight_data, dtype=np.float16)
    wcols = nchunks * WCHUNK * 32
    wimg = np.zeros((128, wcols), dtype=np.float16)
    worder = [np.full(n, -1, dtype=np.int64) for n in nq]
    for e in emitted:
        worder[e["r"]][e["q"]] = e["b"]
    for r in range(4):
        idxs = worder[r]
        nqr = len(idxs)
        blk = np.zeros((nqr, BS, BS), dtype=np.float16)
        sel = np.nonzero(idxs >= 0)[0]
        if len(sel):
            blk[sel] = wd[idxs[sel]].transpose(0, 2, 1)
        wimg[32 * r:32 * r + 32, :nqr * 32] = (
            blk.transpose(1, 0, 2).reshape(BS, nqr * 32))

    xf = np.asarray(x, dtype=np.float32)
    xt_all = np.ascontiguousarray(
        xf.reshape(NCORES, TPC, 32, 4, 32).transpose(0, 3, 4, 2, 1)
    ).astype(np.float16).reshape(NCORES, 128, 32 * TPC)

    bias_img = np.ascontiguousarray(
        np.asarray(bias, dtype=np.float32).reshape(NJ, 128).T)

    return [
        {"xt": xt_all[i], "wimg": wimg, "bias_img": bias_img}
        for i in range(NCORES)
    ]


def _assemble(results):
    out = np.empty((TOKENS, OUT), dtype=np.float32)
    for i, res in enumerate(results):
        out[i * TPC:(i + 1) * TPC] = res["outT"].T
    return out


def _get(block_rows, block_cols):
    if "nc" not in _CACHE:
        chains = _schedule(np.asarray(block_rows), np.asarray(block_cols))
        emitted, nq, nchunks, cdj = _emission(chains)
        nc = _build(emitted, nchunks, cdj)
        _CACHE["nc"] = (nc, emitted, nq, nchunks)
    return _CACHE["nc"]


def kernel(x, weight_data, bias, block_rows, block_cols):
    nc, emitted, nq, nchunks = _get(block_rows, block_cols)
    in_maps = _pack(x, weight_data, bias, emitted, nq, nchunks)
    res = run_bass_kernel_spmd(nc, in_maps, core_ids=list(range(NCORES)))
    return _assemble(res.results)
